# revision 15
# baseline (speedup 1.0000x reference)
"""Temporal GCN (segment-sum message passing) + LSTM on 8 Trainium2
NeuronCores.

Contract: kernel(**inputs) takes the FULL unsharded inputs (same keys as
setup_inputs()) and returns the FULL [T, N, H] float32 output.

Strategy (hardcoded for T=12, N=20000, E=640000, F=128, H=64, 8 cores):
  - Nodes sharded 8 ways by dst (2500/core, padded to 2560 = 20 blocks
    of 128). Host prep: per (t, core, dst-block) edge lists sorted by
    dst (capacity NCB chunks of 128 edges, trailing pads use idx=-1 so
    the SWDGE Q7 skips them), per-chunk dst-local and dinv[dst] scalars,
    x pre-scaled by dinv and pre-transposed to feature-major fp16.
  - On device per timestep:
    Stage A: h'' rows (node-major, fp16, 256B with zero pad) = per
      128-node block one matmul lhsT=x^T-block rhs=W_gcn -> DRAM table.
    Stage B: per dst-block one dma_gather (queues 0-3 round-robin for
      4x Q7 descriptor-gen parallelism) fetches the 128-edge chunks
      token-major; a single DVE tensor_scalar builds the dispatch
      matrix D = (iota == dst_local) * dinv_dst; PE accumulates
      msgs.T @ D into a per-block PSUM tile (the whole segment-sum).
    Stage C: ACT relu(psum + b_gcn) writes the LSTM input feature-major.
    Stage D: LSTM step (PE matmuls + ACT sigmoid/tanh + DVE state
      update); h_t DMA'd out feature-major fp16; host transposes.
"""
import math
import os
import sys

# The kernel needs the axon/neuron jax platform; undo a CPU pin inherited
# from a caller that ran the jax reference first (must happen before jax
# is first imported in this process).
if os.environ.get("JAX_PLATFORMS") == "cpu" and "jax" not in sys.modules:
    del os.environ["JAX_PLATFORMS"]

sys.path.insert(0, "/opt/trn_rl_repo")

import numpy as np

import concourse.bass as bass
import concourse.bacc as bacc
import concourse.mybir as mybir
import concourse.tile as tile
from concourse.library_config import mlp as mlp_lib
from concourse.bass_utils import run_bass_kernel_spmd

FP32 = mybir.dt.float32
FP16 = mybir.dt.float16
I16 = mybir.dt.int16
I32 = mybir.dt.int32
AF = mybir.ActivationFunctionType
OP = mybir.AluOpType

# ---- problem constants (hardcoded per contract)
T, N, E, F, H = 12, 20000, 640000, 128, 64
NCORES = 8
NLOC = N // NCORES              # 2500
NP = (NLOC + 127) // 128 * 128  # 2560
NB = NP // 128                  # 20 dst blocks per core
G = NCORES * NP                 # 20480 rows in the h'' table
G4 = 4 * H
NCB = 36                        # chunk capacity per dst block (128 each)
XCOLS = 5120                    # stage-A x^T DMA chunk (columns)
WSTG = 20                       # stage-A blocks per DRAM write
LSTM_CHUNK = 512
NQ = 4                          # SWDGE queues
USE_REG_COUNTS = False


# ------------------------------------------------------------- host prep

def _host_prep(x, edge_index, W_gcn, b_gcn, W_ih, W_hh, b_ih, b_hh):
    x = np.asarray(x, dtype=np.float32)
    ei = np.asarray(edge_index)

    idxs = np.zeros((NCORES, T, NB, 128, NCB * 8), dtype=np.int16)
    gcnt = np.zeros((NCORES, T, NB), dtype=np.int32)
    dsc = np.zeros((NCORES, T, 128, NB, 2 * NCB), dtype=np.float32)
    dinv_t = np.zeros((T, N), dtype=np.float32)

    loops = np.arange(N, dtype=np.int64)
    for t in range(T):
        src = ei[t, 0].astype(np.int64)
        dst = ei[t, 1].astype(np.int64)
        deg = np.bincount(dst, minlength=N).astype(np.float64) + 1.0
        dinv = 1.0 / np.sqrt(deg)
        dinv_t[t] = dinv
        s_all = np.concatenate([src, loops])
        d_all = np.concatenate([dst, loops])
        order = np.argsort(d_all, kind="stable")
        s_s = s_all[order]
        d_s = d_all[order]
        gb = (d_s // NLOC) * NB + (d_s % NLOC) // 128
        cnt = np.bincount(gb, minlength=NCORES * NB)
        if cnt.max() > NCB * 128:
            raise RuntimeError(f"dst block overflow: {cnt.max()} > {NCB*128}")
        starts = np.concatenate([[0], np.cumsum(cnt)])
        srow = ((s_s // NLOC) * NP + (s_s % NLOC)).astype(np.int16)
        dl = ((d_s % NLOC) % 128).astype(np.float32)
        dv = dinv[d_s].astype(np.float32)
        for c in range(NCORES):
            for b in range(NB):
                g = c * NB + b
                lo, hi = int(starts[g]), int(starts[g + 1])
                k = hi - lo
                gcnt[c, t, b] = k
                idx_flat = np.full(NCB * 128, -1 if USE_REG_COUNTS else 0,
                                   dtype=np.int16)
                idx_flat[:k] = srow[lo:hi]
                dl_flat = np.full(NCB * 128, -1.0, dtype=np.float32)
                dl_flat[:k] = dl[lo:hi]
                dv_flat = np.zeros(NCB * 128, dtype=np.float32)
                dv_flat[:k] = dv[lo:hi]
                idxs[c, t, b] = np.tile(
                    idx_flat.reshape(NCB * 8, 16).T, (8, 1))
                dsc[c, t, :, b, 0:NCB] = dl_flat.reshape(NCB, 128).T
                dsc[c, t, :, b, NCB:] = dv_flat.reshape(NCB, 128).T

    # x pre-scaled by dinv, padded to G columns, feature-major fp16
    xpad = np.zeros((T, G, F), dtype=np.float32)
    dpad = np.zeros((T, G, 1), dtype=np.float32)
    for c in range(NCORES):
        xpad[:, c * NP:c * NP + NLOC] = x[:, c * NLOC:(c + 1) * NLOC]
        dpad[:, c * NP:c * NP + NLOC, 0] = dinv_t[:, c * NLOC:(c + 1) * NLOC]
    xst = np.ascontiguousarray(
        (xpad * dpad).transpose(0, 2, 1)).astype(np.float16)

    iota = np.broadcast_to(np.arange(128, dtype=np.float16), (128, 128))

    common = {
        "xst": xst,
        "iota": np.ascontiguousarray(iota),
        "w_gcn": np.ascontiguousarray(np.asarray(W_gcn), dtype=np.float32),
        "w_ihT": np.ascontiguousarray(np.asarray(W_ih).T, dtype=np.float32),
        "w_hhT": np.ascontiguousarray(np.asarray(W_hh).T, dtype=np.float32),
        "b_ih": np.asarray(b_ih, dtype=np.float32).reshape(-1),
        "b_hh": np.asarray(b_hh, dtype=np.float32).reshape(-1),
        "b_gcn": np.asarray(b_gcn, dtype=np.float32).reshape(-1),
    }
    return [dict(common, idxs=idxs[c], dsc=dsc[c], gcnt=gcnt[c])
            for c in range(NCORES)]


# ------------------------------------------------------------- builder

def _build(reps=1):
    NXC = G // XCOLS            # stage-A x^T chunks per t
    BPC = XCOLS // 128          # blocks per chunk
    NCH = math.ceil(NP / LSTM_CHUNK)

    nc = bacc.Bacc("TRN2", target_bir_lowering=False, debug=False,
                   num_devices=NCORES, num_swdge_queues=NQ)
    xst_ext = nc.dram_tensor("xst", [T, F, G], FP16, kind="ExternalInput").ap()
    idx_ext = nc.dram_tensor("idxs", [T, NB, 128, NCB * 8], I16,
                             kind="ExternalInput").ap()
    dsc_ext = nc.dram_tensor("dsc", [T, 128, NB, 2 * NCB], FP32,
                             kind="ExternalInput").ap()
    iota_ext = nc.dram_tensor("iota", [128, 128], FP16,
                              kind="ExternalInput").ap()
    gcnt_ext = nc.dram_tensor("gcnt", [T, NB], I32, kind="ExternalInput").ap()
    wg_ext = nc.dram_tensor("w_gcn", [F, H], FP32, kind="ExternalInput").ap()
    wih_ext = nc.dram_tensor("w_ihT", [H, G4], FP32, kind="ExternalInput").ap()
    whh_ext = nc.dram_tensor("w_hhT", [H, G4], FP32, kind="ExternalInput").ap()
    bih_ext = nc.dram_tensor("b_ih", [G4], FP32, kind="ExternalInput").ap()
    bhh_ext = nc.dram_tensor("b_hh", [G4], FP32, kind="ExternalInput").ap()
    bg_ext = nc.dram_tensor("b_gcn", [H], FP32, kind="ExternalInput").ap()
    ys_ext = nc.dram_tensor("ys", [T, H, NP], FP16, kind="ExternalOutput").ap()

    hfull = [nc.dram_tensor(f"hfull{t}", [G, F], FP16).ap() for t in range(T)]

    with tile.TileContext(nc) as tc:
        with tc.tile_pool(name="const", bufs=1) as const, \
             tc.tile_pool(name="xp", bufs=2) as xp, \
             tc.tile_pool(name="stgp", bufs=2) as stgp, \
             tc.tile_pool(name="idxp", bufs=6) as idxp, \
             tc.tile_pool(name="dscp", bufs=2) as dscp, \
             tc.tile_pool(name="slabp", bufs=6) as slabp, \
             tc.tile_pool(name="dp", bufs=8) as dp, \
             tc.tile_pool(name="utp", bufs=2) as utp, \
             tc.tile_pool(name="dvp", bufs=2) as dvp, \
             tc.tile_pool(name="ps_a", bufs=2, space="PSUM") as ps_a, \
             tc.tile_pool(name="ps_d", bufs=4, space="PSUM") as ps_d, \
             tc.tile_pool(name="ps_g", bufs=1, space="PSUM") as ps_g:

            nc.gpsimd.load_library(mlp_lib)
            iota_sb = const.tile([128, 128], FP16)
            nc.sync.dma_start(out=iota_sb[:], in_=iota_ext[:])
            wg_sb = const.tile([F, H], FP16)
            nc.gpsimd.dma_start(out=wg_sb[:], in_=wg_ext[:])
            wih_sb = const.tile([H, G4], FP16)
            nc.gpsimd.dma_start(out=wih_sb[:], in_=wih_ext[:])
            whh_sb = const.tile([H, G4], FP16)
            nc.gpsimd.dma_start(out=whh_sb[:], in_=whh_ext[:])
            bsl = G4 // 128
            bih_sb = const.tile([128, bsl], FP32)
            nc.sync.dma_start(out=bih_sb[:],
                              in_=bih_ext.rearrange("(s p) -> p s", p=128))
            bhh_sb = const.tile([128, bsl], FP32)
            nc.sync.dma_start(out=bhh_sb[:],
                              in_=bhh_ext.rearrange("(s p) -> p s", p=128))
            badd = const.tile([128, bsl], FP32)
            nc.vector.tensor_add(out=badd[:], in0=bih_sb[:], in1=bhh_sb[:])
            bg_col = const.tile([H, 1], FP32)
            nc.sync.dma_start(out=bg_col[:], in_=bg_ext[:, None])

            c_sb = const.tile([H, NP], FP32, tag="c_state")
            h16 = const.tile([H, NP], FP16, tag="h_state")
            kreg = nc.gpsimd.alloc_register("gcnt_reg")

            def stage_a(t):
                # Stage A: h'' = (x*dinv) @ W_gcn, node-major fp16 rows
                for xc in range(NXC):
                    xs = xp.tile([128, XCOLS], FP16, tag="xs")
                    nc.sync.dma_start(
                        out=xs[:],
                        in_=xst_ext[t, :, xc * XCOLS:(xc + 1) * XCOLS])
                    for w in range(BPC // WSTG):
                        stg = stgp.tile([128, WSTG, F], FP16, tag="hstage")
                        nc.vector.memset(stg[:, :, H:F], 0.0)
                        for s in range(WSTG):
                            sb = w * WSTG + s
                            h_ps = ps_a.tile([128, H], FP32, space="PSUM",
                                             tag="psa")
                            nc.tensor.matmul(
                                out=h_ps[:],
                                lhsT=xs[:, sb * 128:(sb + 1) * 128],
                                rhs=wg_sb[:], start=True, stop=True)
                            nc.scalar.activation(out=stg[:, s, 0:H],
                                                 in_=h_ps[:], func=AF.Copy)
                        r0 = (xc * BPC + w * WSTG) * 128
                        nc.sync.dma_start(
                            out=hfull[t][r0:r0 + WSTG * 128, :]
                            .rearrange("(s p) f -> p s f", p=128),
                            in_=stg[:])

            for slot in range(reps * T + 1):
                if slot < reps * T:
                    stage_a(slot % T)
                if slot == 0:
                    continue
                t = (slot - 1) % T
                if t == 0:
                    nc.vector.memset(c_sb[:], 0.0)
                    nc.vector.memset(h16[:], 0.0)

                # Stage B/C: gather chunks, dispatch-accumulate, relu
                dsc_sb = dscp.tile([128, NB, 2 * NCB], FP32, tag="dsc")
                nc.sync.dma_start(out=dsc_sb[:], in_=dsc_ext[t])
                cnt_sb = dscp.tile([1, NB], I32, tag="cnt")
                nc.sync.dma_start(out=cnt_sb[:], in_=gcnt_ext[t][None, :])
                uT = utp.tile([H, NP], FP16, tag="uT")
                for b in range(NB):
                    idx_sb = idxp.tile([128, NCB * 8], I16, tag="idx")
                    nc.sync.dma_start(out=idx_sb[:], in_=idx_ext[t, b])
                    slab = slabp.tile([128, NCB, F], FP16, tag="slab")
                    if USE_REG_COUNTS:
                        nc.vector.memset(slab[:], 0.0)
                        nc.gpsimd.reg_load(kreg, cnt_sb[0:1, b:b + 1])
                        nidx_arg = kreg
                    else:
                        nidx_arg = NCB * 128
                    nc.gpsimd.dma_gather(slab[:], hfull[t][:, :], idx_sb[:],
                                         NCB * 128, nidx_arg, F,
                                         single_packet=False,
                                         queue_num=b % NQ)
                    acc_ps = ps_d.tile([128, 128], FP32, space="PSUM",
                                       tag="psd")
                    for ci in range(NCB):
                        D = dp.tile([128, 128], FP16, tag="D")
                        nc.vector.tensor_scalar(
                            out=D[:], in0=iota_sb[:],
                            scalar1=dsc_sb[:, b, ci:ci + 1],
                            scalar2=dsc_sb[:, b, NCB + ci:NCB + ci + 1],
                            op0=OP.is_equal, op1=OP.mult)
                        nc.tensor.matmul(out=acc_ps[:],
                                         lhsT=slab[:, ci, :], rhs=D[:],
                                         start=(ci == 0), stop=(ci == NCB - 1))
                    nc.scalar.activation(out=uT[:, b * 128:(b + 1) * 128],
                                         in_=acc_ps[0:H, :], func=AF.Relu,
                                         bias=bg_col[:])

                # Stage D: LSTM step
                for chi in range(NCH):
                    c0 = chi * LSTM_CHUNK
                    c1 = min(NP, c0 + LSTM_CHUNK)
                    w = c1 - c0
                    ps_if = ps_g.tile([128, LSTM_CHUNK], FP32, space="PSUM",
                                      tag="psif")
                    nc.tensor.matmul(out=ps_if[:, :w], lhsT=wih_sb[:, 0:128],
                                     rhs=uT[:, c0:c1], start=True, stop=False)
                    nc.tensor.matmul(out=ps_if[:, :w], lhsT=whh_sb[:, 0:128],
                                     rhs=h16[:, c0:c1], start=False, stop=True)
                    ps_go = ps_g.tile([128, LSTM_CHUNK], FP32, space="PSUM",
                                      tag="psgo")
                    nc.tensor.matmul(out=ps_go[:, :w], lhsT=wih_sb[:, 128:G4],
                                     rhs=uT[:, c0:c1], start=True, stop=False)
                    nc.tensor.matmul(out=ps_go[:, :w], lhsT=whh_sb[:, 128:G4],
                                     rhs=h16[:, c0:c1], start=False, stop=True)
                    sig_i = dvp.tile([H, LSTM_CHUNK], FP32, tag="sigi")
                    nc.scalar.activation(out=sig_i[:, :w], in_=ps_if[0:H, :w],
                                         func=AF.Sigmoid, bias=badd[0:H, 0:1])
                    sig_f = dvp.tile([H, LSTM_CHUNK], FP32, tag="sigf")
                    nc.scalar.activation(out=sig_f[:, :w], in_=ps_if[H:128, :w],
                                         func=AF.Sigmoid, bias=badd[H:128, 0:1])
                    tanh_g = dvp.tile([H, LSTM_CHUNK], FP32, tag="tanhg")
                    nc.scalar.activation(out=tanh_g[:, :w], in_=ps_go[0:H, :w],
                                         func=AF.Tanh, bias=badd[0:H, 1:2])
                    sig_o = dvp.tile([H, LSTM_CHUNK], FP32, tag="sigo")
                    nc.scalar.activation(out=sig_o[:, :w], in_=ps_go[H:128, :w],
                                         func=AF.Sigmoid, bias=badd[H:128, 1:2])
                    tmp1 = dvp.tile([H, LSTM_CHUNK], FP32, tag="tmp1")
                    nc.vector.tensor_mul(out=tmp1[:, :w], in0=sig_f[:, :w],
                                         in1=c_sb[:, c0:c1])
                    tmp2 = dvp.tile([H, LSTM_CHUNK], FP32, tag="tmp2")
                    nc.vector.tensor_mul(out=tmp2[:, :w], in0=sig_i[:, :w],
                                         in1=tanh_g[:, :w])
                    nc.vector.tensor_add(out=c_sb[:, c0:c1], in0=tmp1[:, :w],
                                         in1=tmp2[:, :w])
                    tanh_c = dvp.tile([H, LSTM_CHUNK], FP32, tag="tanhc")
                    nc.scalar.activation(out=tanh_c[:, :w], in_=c_sb[:, c0:c1],
                                         func=AF.Tanh)
                    nc.vector.tensor_mul(out=h16[:, c0:c1], in0=sig_o[:, :w],
                                         in1=tanh_c[:, :w])
                nc.sync.dma_start(out=ys_ext[t], in_=h16[:])

    nc.compile()
    return nc


_NC_CACHE = {}


def kernel(x, edge_index, W_gcn, b_gcn, W_ih, W_hh, b_ih, b_hh, reps=1):
    in_maps = _host_prep(x, edge_index, W_gcn, b_gcn, W_ih, W_hh, b_ih, b_hh)
    if reps not in _NC_CACHE:
        _NC_CACHE[reps] = _build(reps)
    nc = _NC_CACHE[reps]
    res = run_bass_kernel_spmd(nc, in_maps, core_ids=list(range(NCORES)))
    out = np.empty((T, N, H), dtype=np.float32)
    for c in range(NCORES):
        ys = res.results[c]["ys"]  # [T, H, NP] fp16
        out[:, c * NLOC:(c + 1) * NLOC, :] = \
            ys[:, :, :NLOC].astype(np.float32).transpose(0, 2, 1)
    return out


# revision 16
# speedup vs baseline: 2.4481x; 2.4481x over previous
"""Temporal GCN (segment-sum message passing) + LSTM on 8 Trainium2
NeuronCores.

Contract: kernel(**inputs) takes the FULL unsharded inputs (same keys as
setup_inputs()) and returns the FULL [T, N, H] float32 output.

Strategy (hardcoded for T=12, N=20000, E=640000, F=128, H=64, 8 cores):
  - Nodes sharded 8 ways by dst (2500/core, padded to 2560 = 20 blocks
    of 128). Host prep: per (t, core, dst-block) edge lists sorted by
    dst (capacity NCB chunks of 128 edges, trailing pads use idx=-1 so
    the SWDGE Q7 skips them), per-chunk dst-local and dinv[dst] scalars,
    x pre-scaled by dinv and pre-transposed to feature-major fp16.
  - On device per timestep:
    Stage A: h'' rows (node-major, fp16, 256B with zero pad) = per
      128-node block one matmul lhsT=x^T-block rhs=W_gcn -> DRAM table.
    Stage B: per dst-block one dma_gather (queues 0-3 round-robin for
      4x Q7 descriptor-gen parallelism) fetches the 128-edge chunks
      token-major; a single DVE tensor_scalar builds the dispatch
      matrix D = (iota == dst_local) * dinv_dst; PE accumulates
      msgs.T @ D into a per-block PSUM tile (the whole segment-sum).
    Stage C: ACT relu(psum + b_gcn) writes the LSTM input feature-major.
    Stage D: LSTM step (PE matmuls + ACT sigmoid/tanh + DVE state
      update); h_t DMA'd out feature-major fp16; host transposes.
"""
import math
import os
import sys

# The kernel needs the axon/neuron jax platform; undo a CPU pin inherited
# from a caller that ran the jax reference first (must happen before jax
# is first imported in this process).
if os.environ.get("JAX_PLATFORMS") == "cpu" and "jax" not in sys.modules:
    del os.environ["JAX_PLATFORMS"]

sys.path.insert(0, "/opt/trn_rl_repo")

import numpy as np

import concourse.bass as bass
import concourse.bacc as bacc
import concourse.mybir as mybir
import concourse.tile as tile
from concourse.library_config import mlp as mlp_lib
from concourse.bass_utils import run_bass_kernel_spmd

FP32 = mybir.dt.float32
FP16 = mybir.dt.float16
I16 = mybir.dt.int16
I32 = mybir.dt.int32
AF = mybir.ActivationFunctionType
OP = mybir.AluOpType

# ---- problem constants (hardcoded per contract)
T, N, E, F, H = 12, 20000, 640000, 128, 64
NCORES = 8
NLOC = N // NCORES              # 2500
NP = (NLOC + 127) // 128 * 128  # 2560
NB = NP // 128                  # 20 dst blocks per core
G = NCORES * NP                 # 20480 rows in the h'' table
G4 = 4 * H
NCB = 36                        # chunk capacity per dst block (128 each)
XCOLS = 5120                    # stage-A x^T DMA chunk (columns)
WSTG = 20                       # stage-A blocks per DRAM write
LSTM_CHUNK = 512
NQ = 4                          # SWDGE queues
NSUB = 9                        # sub-gathers per block
SUBI = NCB * 128 // NSUB        # 512 idxs per sub-gather


# ------------------------------------------------------------- host prep

def _host_prep(x, edge_index, W_gcn, b_gcn, W_ih, W_hh, b_ih, b_hh):
    x = np.asarray(x, dtype=np.float32)
    ei = np.asarray(edge_index)

    idxs = np.zeros((NCORES, T, NB, 128, NCB * 8), dtype=np.int16)
    gcnt = np.zeros((NCORES, T, NB), dtype=np.int32)
    dsc = np.zeros((NCORES, T, 128, NB, 2 * NCB), dtype=np.float32)
    dinv_t = np.zeros((T, N), dtype=np.float32)

    loops = np.arange(N, dtype=np.int64)
    for t in range(T):
        src = ei[t, 0].astype(np.int64)
        dst = ei[t, 1].astype(np.int64)
        deg = np.bincount(dst, minlength=N).astype(np.float64) + 1.0
        dinv = 1.0 / np.sqrt(deg)
        dinv_t[t] = dinv
        s_all = np.concatenate([src, loops])
        d_all = np.concatenate([dst, loops])
        order = np.argsort(d_all, kind="stable")
        s_s = s_all[order]
        d_s = d_all[order]
        gb = (d_s // NLOC) * NB + (d_s % NLOC) // 128
        cnt = np.bincount(gb, minlength=NCORES * NB)
        if cnt.max() > NCB * 128:
            raise RuntimeError(f"dst block overflow: {cnt.max()} > {NCB*128}")
        starts = np.concatenate([[0], np.cumsum(cnt)])
        srow = ((s_s // NLOC) * NP + (s_s % NLOC)).astype(np.int16)
        dl = ((d_s % NLOC) % 128).astype(np.float32)
        dv = dinv[d_s].astype(np.float32)
        for c in range(NCORES):
            for b in range(NB):
                g = c * NB + b
                lo, hi = int(starts[g]), int(starts[g + 1])
                k = hi - lo
                gcnt[c, t, b] = k
                idx_flat = np.zeros(NCB * 128, dtype=np.int16)
                idx_flat[:k] = srow[lo:hi]
                dl_flat = np.full(NCB * 128, -1.0, dtype=np.float32)
                dl_flat[:k] = dl[lo:hi]
                dv_flat = np.zeros(NCB * 128, dtype=np.float32)
                dv_flat[:k] = dv[lo:hi]
                idxs[c, t, b] = np.tile(
                    idx_flat.reshape(NCB * 8, 16).T, (8, 1))
                dsc[c, t, :, b, 0:NCB] = dl_flat.reshape(NCB, 128).T
                dsc[c, t, :, b, NCB:] = dv_flat.reshape(NCB, 128).T

    # x pre-scaled by dinv, padded to G columns, feature-major fp16
    xpad = np.zeros((T, G, F), dtype=np.float32)
    dpad = np.zeros((T, G, 1), dtype=np.float32)
    for c in range(NCORES):
        xpad[:, c * NP:c * NP + NLOC] = x[:, c * NLOC:(c + 1) * NLOC]
        dpad[:, c * NP:c * NP + NLOC, 0] = dinv_t[:, c * NLOC:(c + 1) * NLOC]
    xst = np.ascontiguousarray(
        (xpad * dpad).transpose(0, 2, 1)).astype(np.float16)

    iota = np.broadcast_to(np.arange(128, dtype=np.float16), (128, 128))

    common = {
        "xst": xst,
        "iota": np.ascontiguousarray(iota),
        "w_gcn": np.ascontiguousarray(np.asarray(W_gcn), dtype=np.float32),
        "w_ihT": np.ascontiguousarray(np.asarray(W_ih).T, dtype=np.float32),
        "w_hhT": np.ascontiguousarray(np.asarray(W_hh).T, dtype=np.float32),
        "b_ih": np.asarray(b_ih, dtype=np.float32).reshape(-1),
        "b_hh": np.asarray(b_hh, dtype=np.float32).reshape(-1),
        "b_gcn": np.asarray(b_gcn, dtype=np.float32).reshape(-1),
    }
    return [dict(common, idxs=idxs[c], dsc=dsc[c], gcnt=gcnt[c])
            for c in range(NCORES)]


# ------------------------------------------------------------- builder

def _build(reps=1):
    NXC = G // XCOLS            # stage-A x^T chunks per t
    BPC = XCOLS // 128          # blocks per chunk
    NCH = math.ceil(NP / LSTM_CHUNK)

    nc = bacc.Bacc("TRN2", target_bir_lowering=False, debug=False,
                   num_devices=NCORES, num_swdge_queues=NQ)
    xst_ext = nc.dram_tensor("xst", [T, F, G], FP16, kind="ExternalInput").ap()
    idx_ext = nc.dram_tensor("idxs", [T, NB, 128, NCB * 8], I16,
                             kind="ExternalInput").ap()
    dsc_ext = nc.dram_tensor("dsc", [T, 128, NB, 2 * NCB], FP32,
                             kind="ExternalInput").ap()
    iota_ext = nc.dram_tensor("iota", [128, 128], FP16,
                              kind="ExternalInput").ap()
    gcnt_ext = nc.dram_tensor("gcnt", [T, NB], I32, kind="ExternalInput").ap()
    wg_ext = nc.dram_tensor("w_gcn", [F, H], FP32, kind="ExternalInput").ap()
    wih_ext = nc.dram_tensor("w_ihT", [H, G4], FP32, kind="ExternalInput").ap()
    whh_ext = nc.dram_tensor("w_hhT", [H, G4], FP32, kind="ExternalInput").ap()
    bih_ext = nc.dram_tensor("b_ih", [G4], FP32, kind="ExternalInput").ap()
    bhh_ext = nc.dram_tensor("b_hh", [G4], FP32, kind="ExternalInput").ap()
    bg_ext = nc.dram_tensor("b_gcn", [H], FP32, kind="ExternalInput").ap()
    ys_ext = nc.dram_tensor("ys", [T, H, NP], FP16, kind="ExternalOutput").ap()

    hfull = [nc.dram_tensor(f"hfull{t}", [G, F], FP16).ap() for t in range(T)]

    with tile.TileContext(nc) as tc:
        with tc.tile_pool(name="const", bufs=1) as const, \
             tc.tile_pool(name="xp", bufs=2) as xp, \
             tc.tile_pool(name="stgp", bufs=2) as stgp, \
             tc.tile_pool(name="idxp", bufs=6) as idxp, \
             tc.tile_pool(name="dscp", bufs=2) as dscp, \
             tc.tile_pool(name="slabp", bufs=6) as slabp, \
             tc.tile_pool(name="dp", bufs=8) as dp, \
             tc.tile_pool(name="utp", bufs=2) as utp, \
             tc.tile_pool(name="dvp", bufs=2) as dvp, \
             tc.tile_pool(name="ps_a", bufs=2, space="PSUM") as ps_a, \
             tc.tile_pool(name="ps_d", bufs=4, space="PSUM") as ps_d, \
             tc.tile_pool(name="ps_g", bufs=1, space="PSUM") as ps_g:

            nc.gpsimd.load_library(mlp_lib)
            iota_sb = const.tile([128, 128], FP16)
            nc.sync.dma_start(out=iota_sb[:], in_=iota_ext[:])
            wg_sb = const.tile([F, H], FP16)
            nc.gpsimd.dma_start(out=wg_sb[:], in_=wg_ext[:])
            wih_sb = const.tile([H, G4], FP16)
            nc.gpsimd.dma_start(out=wih_sb[:], in_=wih_ext[:])
            whh_sb = const.tile([H, G4], FP16)
            nc.gpsimd.dma_start(out=whh_sb[:], in_=whh_ext[:])
            bsl = G4 // 128
            bih_sb = const.tile([128, bsl], FP32)
            nc.sync.dma_start(out=bih_sb[:],
                              in_=bih_ext.rearrange("(s p) -> p s", p=128))
            bhh_sb = const.tile([128, bsl], FP32)
            nc.sync.dma_start(out=bhh_sb[:],
                              in_=bhh_ext.rearrange("(s p) -> p s", p=128))
            badd = const.tile([128, bsl], FP32)
            nc.vector.tensor_add(out=badd[:], in0=bih_sb[:], in1=bhh_sb[:])
            bg_col = const.tile([H, 1], FP32)
            nc.sync.dma_start(out=bg_col[:], in_=bg_ext[:, None])

            c_sb = const.tile([H, NP], FP32, tag="c_state")
            h16 = const.tile([H, NP], FP16, tag="h_state")

            def stage_a(t):
                # Stage A: h'' = (x*dinv) @ W_gcn, node-major fp16 rows
                for xc in range(NXC):
                    xs = xp.tile([128, XCOLS], FP16, tag="xs")
                    nc.sync.dma_start(
                        out=xs[:],
                        in_=xst_ext[t, :, xc * XCOLS:(xc + 1) * XCOLS])
                    for w in range(BPC // WSTG):
                        stg = stgp.tile([128, WSTG, F], FP16, tag="hstage")
                        nc.vector.memset(stg[:, :, H:F], 0.0)
                        for s in range(WSTG):
                            sb = w * WSTG + s
                            h_ps = ps_a.tile([128, H], FP32, space="PSUM",
                                             tag="psa")
                            nc.tensor.matmul(
                                out=h_ps[:],
                                lhsT=xs[:, sb * 128:(sb + 1) * 128],
                                rhs=wg_sb[:], start=True, stop=True)
                            nc.scalar.activation(out=stg[:, s, 0:H],
                                                 in_=h_ps[:], func=AF.Copy)
                        r0 = (xc * BPC + w * WSTG) * 128
                        nc.sync.dma_start(
                            out=hfull[t][r0:r0 + WSTG * 128, :]
                            .rearrange("(s p) f -> p s f", p=128),
                            in_=stg[:])

            for slot in range(reps * T + 1):
                if slot < reps * T:
                    stage_a(slot % T)
                if slot == 0:
                    continue
                t = (slot - 1) % T
                if t == 0:
                    nc.vector.memset(c_sb[:], 0.0)
                    nc.vector.memset(h16[:], 0.0)

                # Stage B/C: gather chunks, dispatch-accumulate, relu
                dsc_sb = dscp.tile([128, NB, 2 * NCB], FP32, tag="dsc")
                nc.sync.dma_start(out=dsc_sb[:], in_=dsc_ext[t])
                uT = utp.tile([H, NP], FP16, tag="uT")
                for b in range(NB):
                    idx_sb = idxp.tile([128, NCB * 8], I16, tag="idx")
                    nc.sync.dma_start(out=idx_sb[:], in_=idx_ext[t, b])
                    slab = slabp.tile([128, NCB, F], FP16, tag="slab")
                    for k in range(NSUB):
                        s0, s1 = k * (SUBI // 128), (k + 1) * (SUBI // 128)
                        nc.gpsimd.dma_gather(
                            slab[:, s0:s1, :], hfull[t][:, :],
                            idx_sb[:, k * (SUBI // 16):(k + 1) * (SUBI // 16)],
                            SUBI, SUBI, F, single_packet=False,
                            queue_num=(b * NSUB + k) % NQ)
                    acc_ps = ps_d.tile([128, 128], FP32, space="PSUM",
                                       tag="psd")
                    for ci in range(NCB):
                        D = dp.tile([128, 128], FP16, tag="D")
                        nc.vector.tensor_scalar(
                            out=D[:], in0=iota_sb[:],
                            scalar1=dsc_sb[:, b, ci:ci + 1],
                            scalar2=dsc_sb[:, b, NCB + ci:NCB + ci + 1],
                            op0=OP.is_equal, op1=OP.mult)
                        nc.tensor.matmul(out=acc_ps[:],
                                         lhsT=slab[:, ci, :], rhs=D[:],
                                         start=(ci == 0), stop=(ci == NCB - 1))
                    nc.scalar.activation(out=uT[:, b * 128:(b + 1) * 128],
                                         in_=acc_ps[0:H, :], func=AF.Relu,
                                         bias=bg_col[:])

                # Stage D: LSTM step
                for chi in range(NCH):
                    c0 = chi * LSTM_CHUNK
                    c1 = min(NP, c0 + LSTM_CHUNK)
                    w = c1 - c0
                    ps_if = ps_g.tile([128, LSTM_CHUNK], FP32, space="PSUM",
                                      tag="psif")
                    nc.tensor.matmul(out=ps_if[:, :w], lhsT=wih_sb[:, 0:128],
                                     rhs=uT[:, c0:c1], start=True, stop=False)
                    nc.tensor.matmul(out=ps_if[:, :w], lhsT=whh_sb[:, 0:128],
                                     rhs=h16[:, c0:c1], start=False, stop=True)
                    ps_go = ps_g.tile([128, LSTM_CHUNK], FP32, space="PSUM",
                                      tag="psgo")
                    nc.tensor.matmul(out=ps_go[:, :w], lhsT=wih_sb[:, 128:G4],
                                     rhs=uT[:, c0:c1], start=True, stop=False)
                    nc.tensor.matmul(out=ps_go[:, :w], lhsT=whh_sb[:, 128:G4],
                                     rhs=h16[:, c0:c1], start=False, stop=True)
                    sig_i = dvp.tile([H, LSTM_CHUNK], FP32, tag="sigi")
                    nc.scalar.activation(out=sig_i[:, :w], in_=ps_if[0:H, :w],
                                         func=AF.Sigmoid, bias=badd[0:H, 0:1])
                    sig_f = dvp.tile([H, LSTM_CHUNK], FP32, tag="sigf")
                    nc.scalar.activation(out=sig_f[:, :w], in_=ps_if[H:128, :w],
                                         func=AF.Sigmoid, bias=badd[H:128, 0:1])
                    tanh_g = dvp.tile([H, LSTM_CHUNK], FP32, tag="tanhg")
                    nc.scalar.activation(out=tanh_g[:, :w], in_=ps_go[0:H, :w],
                                         func=AF.Tanh, bias=badd[0:H, 1:2])
                    sig_o = dvp.tile([H, LSTM_CHUNK], FP32, tag="sigo")
                    nc.scalar.activation(out=sig_o[:, :w], in_=ps_go[H:128, :w],
                                         func=AF.Sigmoid, bias=badd[H:128, 1:2])
                    tmp1 = dvp.tile([H, LSTM_CHUNK], FP32, tag="tmp1")
                    nc.vector.tensor_mul(out=tmp1[:, :w], in0=sig_f[:, :w],
                                         in1=c_sb[:, c0:c1])
                    tmp2 = dvp.tile([H, LSTM_CHUNK], FP32, tag="tmp2")
                    nc.vector.tensor_mul(out=tmp2[:, :w], in0=sig_i[:, :w],
                                         in1=tanh_g[:, :w])
                    nc.vector.tensor_add(out=c_sb[:, c0:c1], in0=tmp1[:, :w],
                                         in1=tmp2[:, :w])
                    tanh_c = dvp.tile([H, LSTM_CHUNK], FP32, tag="tanhc")
                    nc.scalar.activation(out=tanh_c[:, :w], in_=c_sb[:, c0:c1],
                                         func=AF.Tanh)
                    nc.vector.tensor_mul(out=h16[:, c0:c1], in0=sig_o[:, :w],
                                         in1=tanh_c[:, :w])
                nc.sync.dma_start(out=ys_ext[t], in_=h16[:])

    nc.compile()
    return nc


_NC_CACHE = {}


def kernel(x, edge_index, W_gcn, b_gcn, W_ih, W_hh, b_ih, b_hh, reps=1):
    in_maps = _host_prep(x, edge_index, W_gcn, b_gcn, W_ih, W_hh, b_ih, b_hh)
    if reps not in _NC_CACHE:
        _NC_CACHE[reps] = _build(reps)
    nc = _NC_CACHE[reps]
    res = run_bass_kernel_spmd(nc, in_maps, core_ids=list(range(NCORES)))
    out = np.empty((T, N, H), dtype=np.float32)
    for c in range(NCORES):
        ys = res.results[c]["ys"]  # [T, H, NP] fp16
        out[:, c * NLOC:(c + 1) * NLOC, :] = \
            ys[:, :, :NLOC].astype(np.float32).transpose(0, 2, 1)
    return out
